# revision 13
# baseline (speedup 1.0000x reference)
"""Trainium2 Bass kernel for a single-layer multi-head self-attention.

Math per batch element b (one NeuronCore each):
    z[h] = W[h] @ x_b + b[h]          (d=32, L=1024) per head h in 0..7
    S    = z[h].T @ z[h] / sqrt(d)    (1024, 1024), symmetric since Q=K=V
    out[h] = softmax(S, axis=-1) @ z[h]   -> laid out (d, L) per head

Device layout:
  - z_all (256, 1024): heads stacked along partitions (d-major, group m holds
    heads 4m..4m+3 at partition rows 32s). Feeds score matmuls.
  - ztld  8 x (128, 256): token-major z (zt[tok, 32h+d] = z[h][d, tok]),
    built from 16 PE transposes of z_all blocks (bias rides along).
  - Scores (K=32): 4 heads of a group are packed into the PE array via
    tile_position=(32s, 0) row groups -> 4 concurrent matmuls.
  - E = exp(S/sqrt(d)) unnormalized; symmetric E means row-chunks (l-chunk
    partitions, m free) serve directly as the AV rhs. Row sums are DVE
    tensor_reduce over full (128, 1024) E tiles (keeps ACT free of the
    per-instruction accumulator-readout cost).
  - AV (M=32 per head): 4 heads col-packed via tile_position=(0, 32s) into a
    single (128, 512) PSUM tile per column half -> output lands directly in
    the final (head-stacked) layout.
  - Normalization: reciprocal row-sums are partition-layout (128, 8) tiles;
    DMA roundtrip through DRAM transposes+broadcasts them to (32, 128) column
    slices; one tensor_tensor multiply per half.
"""

import math

import numpy as np

_B, _C, _P, _T = 8, 256, 16, 64
_H, _D = 8, 32
_L = _P * _T  # 1024
_NCORES = 8
_SCALE = 1.0 / math.sqrt(_D)

_CACHE = {}


def _build_nc():
    import concourse.bacc as bacc
    import concourse.mybir as mybir
    import concourse.tile as tile

    fp32 = mybir.dt.float32
    AF = mybir.ActivationFunctionType
    ALU = mybir.AluOpType
    AX = mybir.AxisListType

    nc = bacc.Bacc()

    x_d = nc.dram_tensor("x", [_C, _L], fp32, kind="ExternalInput")
    wt_d = nc.dram_tensor("wt", [_C, _C], fp32, kind="ExternalInput")
    bias_d = nc.dram_tensor("bias", [_C, 1], fp32, kind="ExternalInput")
    ident_d = nc.dram_tensor("ident", [128, 128], fp32, kind="ExternalInput")
    out_d = nc.dram_tensor("out", [_C, _L], fp32, kind="ExternalOutput")

    with tile.TileContext(nc) as tc:
        with (
            tc.tile_pool(name="consts", bufs=1) as consts,
            tc.tile_pool(name="xz", bufs=1) as xz,
            tc.tile_pool(name="epool", bufs=32) as epool,
            tc.tile_pool(name="accp", bufs=8) as accp,
            tc.tile_pool(name="small", bufs=2) as small,
            tc.tile_pool(name="pa", bufs=6, space="PSUM") as pa,
            tc.tile_pool(name="pb", bufs=2, space="PSUM") as pb,
            tc.tile_pool(name="dram", bufs=4, space="DRAM") as dram,
        ):
            # ---- load inputs in first-use order (HWDGE is ~625ns serial per
            # DMA, so the first matmul's operands must head the queue) ----
            xh = [
                [xz.tile([128, 512], fp32, name=f"x{k}_{h}") for h in range(2)]
                for k in range(2)
            ]
            wtc = [
                [consts.tile([128, 128], fp32, name=f"wt{k}_{m}") for m in range(2)]
                for k in range(2)
            ]
            bias_sb = [
                consts.tile([128, 1], fp32, name=f"bias{k}") for k in range(2)
            ]
            ident_sb = consts.tile([128, 128], fp32, name="ident")

            nc.sync.dma_start(xh[0][0], x_d[0:128, 0:512])
            nc.sync.dma_start(wtc[0][0], wt_d[0:128, 0:128])
            nc.sync.dma_start(xh[1][0], x_d[128:256, 0:512])
            nc.sync.dma_start(wtc[1][0], wt_d[128:256, 0:128])
            nc.sync.dma_start(xh[0][1], x_d[0:128, 512:1024])
            nc.sync.dma_start(xh[1][1], x_d[128:256, 512:1024])
            for k in range(2):
                nc.sync.dma_start(bias_sb[k], bias_d[128 * k : 128 * (k + 1), :])
            nc.sync.dma_start(wtc[0][1], wt_d[0:128, 128:256])
            nc.sync.dma_start(wtc[1][1], wt_d[128:256, 128:256])
            nc.sync.dma_start(ident_sb, ident_d[:, :])

            # ---- z_all (256, 1024): heads stacked, projection + bias ----
            z_sb = []
            for m in range(2):
                zm = xz.tile([128, _L], fp32, name=f"z{m}")
                for half in range(2):
                    zp = pa.tile([128, 512], fp32, name=f"zp{m}_{half}", tag="pa")
                    for k in range(2):
                        nc.tensor.matmul(
                            zp,
                            wtc[k][m],
                            xh[k][half],
                            start=(k == 0),
                            stop=(k == 1),
                        )
                    nc.vector.tensor_scalar_add(
                        zm[:, 512 * half : 512 * (half + 1)], zp, bias_sb[m]
                    )
                z_sb.append(zm)

            # ---- attention state + helpers (ztld transposes are interleaved
            # after the first score round so the PE p-state ramp never resets)
            zt_sb = []
            G = {}

            def group_init(m):
                G[m] = {
                    "avh": [
                        pb.tile([128, 512], fp32, name=f"av{m}_{h}", tag="pb")
                        for h in range(2)
                    ],
                    "rs8": [
                        accp.tile([128, 8], fp32, name=f"rs8{m}_{s}", tag="rs8")
                        for s in range(4)
                    ],
                    "es": {},
                }

            def score_round(m, i):
                zg = z_sb[m]
                es, rs8 = G[m]["es"], G[m]["rs8"]
                for half in range(2):
                    sps = []
                    for s in range(4):
                        sp = pa.tile(
                            [128, 512], fp32, name=f"sp{m}_{i}_{half}_{s}", tag="pa"
                        )
                        nc.tensor.matmul(
                            sp,
                            zg[32 * s : 32 * (s + 1), 128 * i : 128 * (i + 1)],
                            zg[32 * s : 32 * (s + 1), 512 * half : 512 * (half + 1)],
                            start=True,
                            stop=True,
                            tile_position=(32 * s, 0),
                        )
                        sps.append(sp)
                    for s in range(4):
                        if half == 0:
                            es[(s, i)] = epool.tile(
                                [128, _L], fp32, name=f"e{m}_{s}_{i}", tag="e"
                            )
                        nc.scalar.activation(
                            es[(s, i)][:, 512 * half : 512 * (half + 1)],
                            sps[s],
                            AF.Exp,
                            scale=_SCALE,
                        )
                for s in range(4):
                    nc.vector.tensor_reduce(
                        rs8[s][:, i : i + 1], es[(s, i)], axis=AX.X, op=ALU.add
                    )

            def issue_av(m, j, halves=(0, 1)):
                avh, es = G[m]["avh"], G[m]["es"]
                for half in halves:
                    for s in range(4):
                        nc.tensor.matmul(
                            avh[half][32 * s : 32 * (s + 1), :],
                            zt_sb[j][:, 128 * m + 32 * s : 128 * m + 32 * (s + 1)],
                            es[(s, j)][:, 512 * half : 512 * (half + 1)],
                            start=(j == 0),
                            stop=(j == 7),
                            tile_position=(0, 32 * s),
                            skip_group_check=True,
                        )

            def normalize(m):
                rbt = small.tile([128, _L], fp32, name=f"rb{m}", tag="rb")
                G[m]["rbt"] = rbt
                for s in range(4):
                    rrec = small.tile([128, 8], fp32, name=f"rrec{m}_{s}", tag="rrec")
                    nc.vector.reciprocal(rrec, G[m]["rs8"][s])
                    dr = dram.tile([8, 128], fp32, name=f"dr{m}_{s}", tag="dr")
                    # transpose to l-order in DRAM: dr[i, p] = rrec[p, i]
                    nc.sync.dma_start(dr[:, :].rearrange("a b -> b a"), rrec)
                    # broadcast back: rbt[32s+d, 128i+p] = dr[i, p]
                    nc.sync.dma_start(
                        rbt[32 * s : 32 * (s + 1), :].rearrange(
                            "d (i q) -> d i q", i=8
                        ),
                        dr[:, :].unsqueeze(0).to_broadcast([32, 8, 128]),
                    )

            def finish(m):
                o = small.tile([128, _L], fp32, name=f"o{m}", tag="o")
                for half in range(2):
                    issue_av(m, 7, halves=(half,))
                    nc.vector.tensor_tensor(
                        o[:, 512 * half : 512 * (half + 1)],
                        G[m]["avh"][half],
                        G[m]["rbt"][:, 512 * half : 512 * (half + 1)],
                        op=ALU.mult,
                    )
                    nc.sync.dma_start(
                        out_d[
                            128 * m : 128 * (m + 1), 512 * half : 512 * (half + 1)
                        ],
                        o[:, 512 * half : 512 * (half + 1)],
                    )

            # ---- schedule ----
            group_init(0)
            score_round(0, 0)

            # ztld 8 x (128, 256): token-major z via PE transposes (bias rides
            # along from z_all); slotted here so PE stays hot while z settles.
            for j in range(8):
                ztp = pa.tile([128, 512], fp32, name=f"ztp{j}", tag="pa")
                for k in range(2):
                    nc.tensor.transpose(
                        ztp[:, 128 * k : 128 * (k + 1)],
                        z_sb[k][:, 128 * j : 128 * (j + 1)],
                        ident_sb,
                    )
                ztj = xz.tile([128, _C], fp32, name=f"zt{j}")
                nc.vector.tensor_copy(ztj, ztp[:, 0:_C])
                zt_sb.append(ztj)

            for i in range(1, 8):
                score_round(0, i)
                if i <= 6:
                    issue_av(0, i - 1)
            normalize(0)
            issue_av(0, 6)
            finish(0)

            group_init(1)
            for i in range(8):
                score_round(1, i)
                if 1 <= i <= 6:
                    issue_av(1, i - 1)
            normalize(1)
            issue_av(1, 6)
            finish(1)

    nc.finalize()
    return nc


def _get_compiled():
    if "nc" not in _CACHE:
        _CACHE["nc"] = _build_nc()
    return _CACHE["nc"]


def kernel(x: np.ndarray, W: np.ndarray, b: np.ndarray) -> np.ndarray:
    from concourse.bass_utils import run_bass_kernel_spmd

    x = np.ascontiguousarray(x, dtype=np.float32)
    W = np.ascontiguousarray(W, dtype=np.float32)
    b = np.ascontiguousarray(b, dtype=np.float32)

    wt = np.ascontiguousarray(W.reshape(_H * _D, _C).T)  # (C, H*D)
    bias = np.ascontiguousarray(b.reshape(_H * _D, 1))
    ident = np.eye(128, dtype=np.float32)

    in_maps = [
        {
            "x": np.ascontiguousarray(x[i].reshape(_C, _L)),
            "wt": wt,
            "bias": bias,
            "ident": ident,
        }
        for i in range(_NCORES)
    ]

    nc = _get_compiled()
    res = run_bass_kernel_spmd(nc, in_maps, list(range(_NCORES)))
    out = np.stack(
        [res.results[i]["out"].reshape(_H * _D, _P, _T) for i in range(_NCORES)]
    )
    return out


# revision 17
# speedup vs baseline: 1.0087x; 1.0087x over previous
"""Trainium2 Bass kernel for a single-layer multi-head self-attention.

Math per batch element b (one NeuronCore each):
    z[h] = W[h] @ x_b + b[h]          (d=32, L=1024) per head h in 0..7
    S    = z[h].T @ z[h] / sqrt(d)    (1024, 1024), symmetric since Q=K=V
    out[h] = softmax(S, axis=-1) @ z[h]   -> laid out (d, L) per head

Device layout:
  - z_all (256, 1024): heads stacked along partitions (d-major, group m holds
    heads 4m..4m+3 at partition rows 32s). Feeds score matmuls.
  - ztld  8 x (128, 256): token-major z (zt[tok, 32h+d] = z[h][d, tok]),
    built from 16 PE transposes of z_all blocks (bias rides along).
  - Scores (K=32): 4 heads of a group are packed into the PE array via
    tile_position=(32s, 0) row groups -> 4 concurrent matmuls.
  - E = exp(S/sqrt(d)) unnormalized; symmetric E means row-chunks (l-chunk
    partitions, m free) serve directly as the AV rhs. Row sums are DVE
    tensor_reduce over full (128, 1024) E tiles (keeps ACT free of the
    per-instruction accumulator-readout cost).
  - AV (M=32 per head): 4 heads col-packed via tile_position=(0, 32s) into a
    single (128, 512) PSUM tile per column half -> output lands directly in
    the final (head-stacked) layout.
  - Normalization: reciprocal row-sums are partition-layout (128, 8) tiles;
    DMA roundtrip through DRAM transposes+broadcasts them to (32, 128) column
    slices; one tensor_tensor multiply per half.
"""

import math

import numpy as np

_B, _C, _P, _T = 8, 256, 16, 64
_H, _D = 8, 32
_L = _P * _T  # 1024
_NCORES = 8
_SCALE = 1.0 / math.sqrt(_D)

_CACHE = {}


def _build_nc():
    import concourse.bacc as bacc
    import concourse.mybir as mybir
    import concourse.tile as tile

    fp32 = mybir.dt.float32
    AF = mybir.ActivationFunctionType
    ALU = mybir.AluOpType
    AX = mybir.AxisListType

    nc = bacc.Bacc()

    x_d = nc.dram_tensor("x", [_C, _L], fp32, kind="ExternalInput")
    wt_d = nc.dram_tensor("wt", [_C, _C], fp32, kind="ExternalInput")
    bias_d = nc.dram_tensor("bias", [_C, 1], fp32, kind="ExternalInput")
    ident_d = nc.dram_tensor("ident", [128, 128], fp32, kind="ExternalInput")
    out_d = nc.dram_tensor("out", [_C, _L], fp32, kind="ExternalOutput")

    with tile.TileContext(nc) as tc:
        with (
            tc.tile_pool(name="consts", bufs=1) as consts,
            tc.tile_pool(name="xz", bufs=1) as xz,
            tc.tile_pool(name="epool", bufs=32) as epool,
            tc.tile_pool(name="accp", bufs=8) as accp,
            tc.tile_pool(name="small", bufs=2) as small,
            tc.tile_pool(name="pa", bufs=6, space="PSUM") as pa,
            tc.tile_pool(name="pb", bufs=2, space="PSUM") as pb,
            tc.tile_pool(name="dram", bufs=4, space="DRAM") as dram,
        ):
            # ---- load inputs in first-use order (HWDGE is ~625ns serial per
            # DMA, so the first matmul's operands must head the queue) ----
            xh = [
                [xz.tile([128, 512], fp32, name=f"x{k}_{h}") for h in range(2)]
                for k in range(2)
            ]
            wtc = [
                [consts.tile([128, 128], fp32, name=f"wt{k}_{m}") for m in range(2)]
                for k in range(2)
            ]
            bias_sb = [
                consts.tile([128, 1], fp32, name=f"bias{k}") for k in range(2)
            ]
            ident_sb = consts.tile([128, 128], fp32, name="ident")

            nc.sync.dma_start(xh[0][0], x_d[0:128, 0:512])
            nc.sync.dma_start(wtc[0][0], wt_d[0:128, 0:128])
            nc.sync.dma_start(xh[1][0], x_d[128:256, 0:512])
            nc.sync.dma_start(wtc[1][0], wt_d[128:256, 0:128])
            nc.sync.dma_start(xh[0][1], x_d[0:128, 512:1024])
            nc.sync.dma_start(xh[1][1], x_d[128:256, 512:1024])
            for k in range(2):
                nc.sync.dma_start(bias_sb[k], bias_d[128 * k : 128 * (k + 1), :])
            nc.sync.dma_start(wtc[0][1], wt_d[0:128, 128:256])
            nc.sync.dma_start(wtc[1][1], wt_d[128:256, 128:256])
            nc.sync.dma_start(ident_sb, ident_d[:, :])

            # ---- z_all (256, 1024): heads stacked, projection + bias ----
            z_sb = []
            for m in range(2):
                zm = xz.tile([128, _L], fp32, name=f"z{m}")
                for half in range(2):
                    zp = pa.tile([128, 512], fp32, name=f"zp{m}_{half}", tag="pa")
                    for k in range(2):
                        nc.tensor.matmul(
                            zp,
                            wtc[k][m],
                            xh[k][half],
                            start=(k == 0),
                            stop=(k == 1),
                        )
                    nc.vector.tensor_scalar_add(
                        zm[:, 512 * half : 512 * (half + 1)], zp, bias_sb[m]
                    )
                z_sb.append(zm)

            # ---- attention state + helpers (ztld transposes are interleaved
            # after the first score round so the PE p-state ramp never resets)
            zt_sb = []
            G = {}

            def group_init(m):
                G[m] = {
                    "avh": [
                        pb.tile([128, 512], fp32, name=f"av{m}_{h}", tag="pb")
                        for h in range(2)
                    ],
                    "rs8": [
                        accp.tile([128, 8], fp32, name=f"rs8{m}_{s}", tag="rs8")
                        for s in range(4)
                    ],
                    "es": {},
                }

            def issue_reduces(m, i):
                es, rs8 = G[m]["es"], G[m]["rs8"]
                for s in range(4):
                    nc.vector.tensor_reduce(
                        rs8[s][:, i : i + 1], es[(s, i)], axis=AX.X, op=ALU.add
                    )

            def score_round(m, i, reduce=True):
                zg = z_sb[m]
                es = G[m]["es"]
                for half in range(2):
                    sps = []
                    for s in range(4):
                        sp = pa.tile(
                            [128, 512], fp32, name=f"sp{m}_{i}_{half}_{s}", tag="pa"
                        )
                        nc.tensor.matmul(
                            sp,
                            zg[32 * s : 32 * (s + 1), 128 * i : 128 * (i + 1)],
                            zg[32 * s : 32 * (s + 1), 512 * half : 512 * (half + 1)],
                            start=True,
                            stop=True,
                            tile_position=(32 * s, 0),
                        )
                        sps.append(sp)
                    for s in range(4):
                        if half == 0:
                            es[(s, i)] = epool.tile(
                                [128, _L], fp32, name=f"e{m}_{s}_{i}", tag="e"
                            )
                        nc.scalar.activation(
                            es[(s, i)][:, 512 * half : 512 * (half + 1)],
                            sps[s],
                            AF.Exp,
                            scale=_SCALE,
                        )
                if reduce:
                    issue_reduces(m, i)

            def issue_av(m, j, halves=(0, 1)):
                avh, es = G[m]["avh"], G[m]["es"]
                for half in halves:
                    for s in range(4):
                        nc.tensor.matmul(
                            avh[half][32 * s : 32 * (s + 1), :],
                            zt_sb[j // 2][
                                :,
                                256 * (j % 2)
                                + 128 * m
                                + 32 * s : 256 * (j % 2)
                                + 128 * m
                                + 32 * (s + 1),
                            ],
                            es[(s, j)][:, 512 * half : 512 * (half + 1)],
                            start=(j == 0),
                            stop=(j == 7),
                            tile_position=(0, 32 * s),
                            skip_group_check=True,
                        )

            def normalize(m):
                rbt = small.tile([128, _L], fp32, name=f"rb{m}", tag="rb")
                G[m]["rbt"] = rbt
                for s in range(4):
                    rrec = small.tile([128, 8], fp32, name=f"rrec{m}_{s}", tag="rrec")
                    nc.vector.reciprocal(rrec, G[m]["rs8"][s])
                    dr = dram.tile([8, 128], fp32, name=f"dr{m}_{s}", tag="dr")
                    # transpose to l-order in DRAM: dr[i, p] = rrec[p, i]
                    nc.sync.dma_start(dr[:, :].rearrange("a b -> b a"), rrec)
                    # broadcast back: rbt[32s+d, 128i+p] = dr[i, p]
                    nc.sync.dma_start(
                        rbt[32 * s : 32 * (s + 1), :].rearrange(
                            "d (i q) -> d i q", i=8
                        ),
                        dr[:, :].unsqueeze(0).to_broadcast([32, 8, 128]),
                    )

            def finish(m):
                o = small.tile([128, _L], fp32, name=f"o{m}", tag="o")
                for half in range(2):
                    issue_av(m, 7, halves=(half,))
                    nc.vector.tensor_tensor(
                        o[:, 512 * half : 512 * (half + 1)],
                        G[m]["avh"][half],
                        G[m]["rbt"][:, 512 * half : 512 * (half + 1)],
                        op=ALU.mult,
                    )
                    nc.sync.dma_start(
                        out_d[
                            128 * m : 128 * (m + 1), 512 * half : 512 * (half + 1)
                        ],
                        o[:, 512 * half : 512 * (half + 1)],
                    )

            # ---- schedule ----
            group_init(0)
            score_round(0, 0, reduce=False)

            # ztld 4 x (128, 512): token-major z via PE transposes (bias rides
            # along from z_all); 4 transposes per PSUM tile so the pa ring
            # never WAW-stalls on a ztp slot, and the copies run on DVE before
            # round-0's reduces (deferred below) to free PSUM promptly.
            for jj in range(4):
                ztp = pa.tile([128, 512], fp32, name=f"ztp{jj}", tag="pa")
                for u in range(2):
                    j = 2 * jj + u
                    for k in range(2):
                        nc.tensor.transpose(
                            ztp[:, 256 * u + 128 * k : 256 * u + 128 * (k + 1)],
                            z_sb[k][:, 128 * j : 128 * (j + 1)],
                            ident_sb,
                        )
                zt2 = xz.tile([128, 512], fp32, name=f"zt2_{jj}")
                nc.vector.tensor_copy(zt2, ztp)
                zt_sb.append(zt2)
            issue_reduces(0, 0)

            for i in range(1, 8):
                score_round(0, i)
                if i <= 6:
                    issue_av(0, i - 1)
            normalize(0)
            issue_av(0, 6)
            finish(0)

            group_init(1)
            for i in range(8):
                score_round(1, i)
                if 1 <= i <= 6:
                    issue_av(1, i - 1)
            normalize(1)
            issue_av(1, 6)
            finish(1)

    nc.finalize()
    return nc


def _get_compiled():
    if "nc" not in _CACHE:
        _CACHE["nc"] = _build_nc()
    return _CACHE["nc"]


def kernel(x: np.ndarray, W: np.ndarray, b: np.ndarray) -> np.ndarray:
    from concourse.bass_utils import run_bass_kernel_spmd

    x = np.ascontiguousarray(x, dtype=np.float32)
    W = np.ascontiguousarray(W, dtype=np.float32)
    b = np.ascontiguousarray(b, dtype=np.float32)

    wt = np.ascontiguousarray(W.reshape(_H * _D, _C).T)  # (C, H*D)
    bias = np.ascontiguousarray(b.reshape(_H * _D, 1))
    ident = np.eye(128, dtype=np.float32)

    in_maps = [
        {
            "x": np.ascontiguousarray(x[i].reshape(_C, _L)),
            "wt": wt,
            "bias": bias,
            "ident": ident,
        }
        for i in range(_NCORES)
    ]

    nc = _get_compiled()
    res = run_bass_kernel_spmd(nc, in_maps, list(range(_NCORES)))
    out = np.stack(
        [res.results[i]["out"].reshape(_H * _D, _P, _T) for i in range(_NCORES)]
    )
    return out
